# revision 44
# baseline (speedup 1.0000x reference)
"""Trainium2 Bass kernel for nn_JointLoss (recon MSE + SimCLR contrastive + group distance loss).

Strategy (symmetric data-parallel over 8 NeuronCores):
  - exp(sim) dominates: ACT is the only engine with exp, 1 col/cycle @1.2GHz.
    The sim matrix is symmetric: each core evaluates its 1024 rows vs column
    groups c..c+3 (full) plus 3 quarters of the c+4 pair block -- 38,912 cols
    instead of 65,536.  Transposed entries are recovered as column sums.
  - Host pre-transposes projections into the fp8e4m3 DoubleRow layout
    [64, 2, 5120] (x16): no on-chip transposes, sim matmuls at 0.5 cyc/col.
  - 24 activations (PSUM regions A/B = [0:1792]/[1792:3584] ping-pong) write
    exp to SBUF bf16 with accum_out row-sums.
  - Column-sum tile-accumulation over [1024:3584] runs as gpsimd
    DMA-accumulate transfers on the otherwise-idle DMA path; the PE then does
    one ones-matmul per 512-col chunk.  The [3584:4096] and [4608:5120]
    pieces accumulate with a few small DVE adds.
  - possum (masked diagonal sums) moves to the host via tiny diag-block DMA
    dumps; recon-MSE is GpSimd subtract+square with DVE reduces; distance
    partials on DVE.  This keeps the DVE stream light -- its real throughput
    is ~2x below the scheduler's model and heavy DVE work causes cross-engine
    stalls.
  - Host finishes in float64 from tiny outputs.
"""

import os
import sys

if "/opt/trn_rl_repo" not in sys.path:
    sys.path.insert(0, "/opt/trn_rl_repo")

from contextlib import ExitStack

import numpy as np
import ml_dtypes

import concourse.bacc as bacc
import concourse.tile as tile
from concourse import mybir
from concourse.bass_utils import run_bass_kernel_spmd

N = 8192
D = 128
F = 784
NCORES = 8
RPC = N // NCORES
RT = RPC // 128
PROJ_ROWS = 5 * RPC
TAU = 0.1
FP8_SCALE = 16.0

f32 = mybir.dt.float32
bf16 = mybir.dt.bfloat16
fp8 = mybir.dt.float8e4
BF = ml_dtypes.bfloat16
F8 = ml_dtypes.float8_e4m3fn

COL_CHUNKS = [
    (1024, 8), (1536, 8), (2048, 8), (2560, 8), (3072, 8),  # from accA
    (3584, 8),                                              # from sa5
    (4608, 4),                                              # from sa6 (tiles 0-3)
]

GB0, GB1, GB2 = 1792, 3584, 5120


def _bank_splits(a, b):
    cuts = [a]
    nxt = (a // 512 + 1) * 512
    while nxt < b:
        cuts.append(nxt)
        nxt += 512
    cuts.append(b)
    return list(zip(cuts[:-1], cuts[1:]))


def _kernel_body(tc, ptdr, pb16, xr, rl, rowsum_o, diag_o, colsum_o, partials_o):
    nc = tc.nc
    AX = mybir.AxisListType
    ALU = mybir.AluOpType
    EXP = mybir.ActivationFunctionType.Exp
    DR = mybir.MatmulPerfMode.DoubleRow
    with ExitStack() as ctx:
        consts = ctx.enter_context(tc.tile_pool(name="consts", bufs=1))
        big = ctx.enter_context(tc.tile_pool(name="big", bufs=1))
        stats = ctx.enter_context(tc.tile_pool(name="stats", bufs=1))
        psum = ctx.enter_context(tc.tile_pool(name="psum", bufs=1, space="PSUM"))

        pt_dr = big.tile([64, 2, PROJ_ROWS], fp8)
        # only the first column chunk up front: the first matmul's dma-sem wait
        # covers everything queued before it, so keep that queue short
        nc.sync.dma_start(pt_dr[:, :, 0:GB0], ptdr[:, :, 0:GB0])
        pb = consts.tile([128, RPC], bf16)

        exp_sbuf = big.tile([128, RT, PROJ_ROWS], bf16)
        xr_sb = big.tile([128, RT, F], bf16)
        rl_sb = big.tile([128, RT, F], bf16)

        ones_bf = consts.tile([128, 1], bf16)
        nc.vector.memset(ones_bf, 1.0)

        rparts = stats.tile([128, RT, 3], f32)
        rowsum_sb = stats.tile([128, RT], f32)
        recon_parts = stats.tile([128, RT], f32)
        stats4 = stats.tile([128, 4], f32)
        s_groups = stats.tile([128, RPC // 4], f32)
        junk_bf = stats.tile([128, RPC], bf16)
        junkf = stats.tile([128, RPC // 4], f32)
        d_bf = stats.tile([128, F], bf16)
        d2_all = stats.tile([128, RT, F], bf16)
        colstage = stats.tile([128, 7, 512], f32)
        accA = stats.tile([128, 2560], bf16)   # tile sums over cols [1024:3584]
        accB = stats.tile([128, 2560], bf16)
        accY = stats.tile([128, 2560], bf16)
        sa5 = stats.tile([128, 512], bf16)
        sb5 = stats.tile([128, 512], bf16)
        y5 = stats.tile([128, 512], bf16)
        sa6 = stats.tile([128, 512], bf16)
        y6 = stats.tile([128, 512], bf16)

        pacc = psum.tile([128, 4096], f32)
        REG = [0, 1792]

        def colsum_mm(src):
            nc.tensor.matmul(
                pacc[0:1, 3584:4096], ones_bf[:, 0:1], src, start=True, stop=True
            )

        def colsum_drain(k):
            nc.vector.tensor_copy(colstage[0:1, k, :], pacc[0:1, 3584:4096])
            nc.sync.dma_start(colsum_o[k], colstage[0:1, k, :])

        def sim_act(t, au, g):
            reg = REG[au % 2]
            w = pt_dr[:, :, t * 128 : (t + 1) * 128]
            if g == 0:
                cols = [(0, GB0)]
            elif g == 1:
                cols = [(GB0, GB1)]
            elif t < 4:
                cols = [(GB1, GB2)]
            else:
                cols = [(GB1, 4096), (4608, 5120)]
            off = 0
            for c0, c1 in cols:
                for p0, p1 in _bank_splits(reg + off, reg + off + (c1 - c0)):
                    s0 = c0 + (p0 - reg - off)
                    nc.tensor.matmul(
                        pacc[:, p0:p1], w, pt_dr[:, :, s0 : s0 + (p1 - p0)],
                        start=True, stop=True, perf_mode=DR,
                    )
                off += c1 - c0
            if g == 0:
                o0, o1 = 0, GB0
            elif g == 1:
                o0, o1 = GB0, GB1
            elif t < 4:
                o0, o1 = GB1, GB2
            else:
                o0, o1 = GB1, 4608
            nc.scalar.activation(
                exp_sbuf[:, t, o0:o1],
                pacc[:, reg : reg + (o1 - o0)],
                EXP,
                scale=1.0 / (TAU * FP8_SCALE * FP8_SCALE),
                accum_out=rparts[:, t, g : g + 1],
            )

        au = 0
        E = lambda t, a, b: exp_sbuf[:, t, a:b]

        # ---- G0 pass: cols [0:1792]; stream remaining input DMAs; diag dumps ----
        for t in range(RT):
            sim_act(t, au, 0)
            au += 1
            nc.sync.dma_start(diag_o[t], exp_sbuf[:, t, t * 128 : (t + 1) * 128])
            if t == 0:
                nc.sync.dma_start(pt_dr[:, :, GB0:GB1], ptdr[:, :, GB0:GB1])
                nc.sync.dma_start(pt_dr[:, :, GB1:GB2], ptdr[:, :, GB1:GB2])
                nc.sync.dma_start(pb, pb16)
                nc.sync.dma_start(xr_sb[:, 0:4, :], xr[0:512].rearrange("(t p) j -> p t j", p=128))
                nc.sync.dma_start(rl_sb[:, 0:4, :], rl[0:512].rearrange("(t p) j -> p t j", p=128))
                nc.sync.dma_start(xr_sb[:, 4:8, :], xr[512:1024].rearrange("(t p) j -> p t j", p=128))
                nc.sync.dma_start(rl_sb[:, 4:8, :], rl[512:1024].rearrange("(t p) j -> p t j", p=128))

        # distance-loss partials (DVE) from own rows
        nc.vector.reduce_sum(s_groups, pb.rearrange("p (g s) -> p g s", s=4), axis=AX.X)
        nc.vector.affine_mul_reduce(
            out=junk_bf, accum_out=stats4[:, 1:2], in0=pb, in1=pb, scale=1.0, bias=0.0
        )
        nc.vector.tensor_tensor(junkf, s_groups, s_groups, ALU.mult)
        nc.vector.reduce_sum(stats4[:, 2:3], junkf, axis=AX.X)
        nc.vector.memset(stats4[:, 3:4], 0.0)

        # ---- G1 pass: cols [1792:3584]; dma-accumulate [1024:3584]; MSE ----
        for t in range(RT):
            sim_act(t, au, 1)
            au += 1
            # DVE tree accumulation of exp tiles over cols [1024:3584]
            if t == 1:
                nc.vector.tensor_tensor(accA, E(0, 1024, 3584), E(1, 1024, 3584), ALU.add)
            if t == 3:
                nc.vector.tensor_tensor(accY, E(2, 1024, 3584), E(3, 1024, 3584), ALU.add)
                nc.vector.tensor_tensor(accA, accA, accY, ALU.add)
            if t == 5:
                nc.vector.tensor_tensor(accB, E(4, 1024, 3584), E(5, 1024, 3584), ALU.add)
            if t == 7:
                nc.vector.tensor_tensor(accY, E(6, 1024, 3584), E(7, 1024, 3584), ALU.add)
                nc.vector.tensor_tensor(accB, accB, accY, ALU.add)
            nc.gpsimd.tensor_tensor(d_bf, xr_sb[:, t, :], rl_sb[:, t, :], ALU.subtract)
            nc.gpsimd.tensor_tensor(d2_all[:, t, :], d_bf, d_bf, ALU.mult)

        # ---- G2 pass: cols [3584:5120] (packed t>=4); colsum mms; MSE reduce ----
        for t in range(RT):
            sim_act(t, au, 2)
            au += 1
            if t == 0:
                nc.vector.tensor_tensor(accA, accA, accB, ALU.add)
            if t == 1:
                nc.vector.tensor_tensor(sa5, E(0, 3584, 4096), E(1, 3584, 4096), ALU.add)
            if t == 3:
                nc.vector.tensor_tensor(y5, E(2, 3584, 4096), E(3, 3584, 4096), ALU.add)
                nc.vector.tensor_tensor(sa5, sa5, y5, ALU.add)
                nc.vector.tensor_tensor(sa6, E(0, 4608, 5120), E(1, 4608, 5120), ALU.add)
                nc.vector.tensor_tensor(y6, E(2, 4608, 5120), E(3, 4608, 5120), ALU.add)
                nc.vector.tensor_tensor(sa6, sa6, y6, ALU.add)
            if t == 5:
                nc.vector.tensor_tensor(sb5, E(4, 3584, 4096), E(5, 3584, 4096), ALU.add)
            if t == 7:
                nc.vector.tensor_tensor(y5, E(6, 3584, 4096), E(7, 3584, 4096), ALU.add)
                nc.vector.tensor_tensor(sb5, sb5, y5, ALU.add)
                nc.vector.tensor_tensor(sa5, sa5, sb5, ALU.add)
            # chunk mms: accA chunks at steps 1-5, chunk6 at t6; each step
            # drains the previous chunk BEFORE the next mm reuses the slot
            if 2 <= t <= 6:
                colsum_drain(t - 2)
            if 1 <= t <= 5:
                colsum_mm(accA[:, (t - 1) * 512 : t * 512])
            if t == 6:
                colsum_mm(sa6)
            if t == 7:
                colsum_drain(6)
            # MSE reduce of tile t
            nc.vector.reduce_sum(recon_parts[:, t : t + 1], d2_all[:, t, :], axis=AX.X)

        # epilogue: chunk 5
        colsum_mm(sa5)
        colsum_drain(5)

        nc.vector.reduce_sum(rowsum_sb, rparts, axis=AX.X)
        nc.vector.reduce_sum(stats4[:, 0:1], recon_parts, axis=AX.X)

        nc.sync.dma_start(rowsum_o, rowsum_sb)
        nc.sync.dma_start(partials_o, stats4)


def _build():
    nc = bacc.Bacc("TRN2", target_bir_lowering=False, debug=False, num_devices=NCORES)
    ptdr = nc.dram_tensor("ptdr", [64, 2, PROJ_ROWS], fp8, kind="ExternalInput").ap()
    pb16 = nc.dram_tensor("pb16", [128, RPC], bf16, kind="ExternalInput").ap()
    xr = nc.dram_tensor("xr", [RPC, F], bf16, kind="ExternalInput").ap()
    rl = nc.dram_tensor("rl", [RPC, F], bf16, kind="ExternalInput").ap()
    rowsum_o = nc.dram_tensor("rowsum_o", [128, RT], f32, kind="ExternalOutput").ap()
    diag_o = nc.dram_tensor("diag_o", [RT, 128, 128], bf16, kind="ExternalOutput").ap()
    colsum_o = nc.dram_tensor("colsum_o", [7, 512], f32, kind="ExternalOutput").ap()
    partials_o = nc.dram_tensor("partials_o", [128, 4], f32, kind="ExternalOutput").ap()

    with tile.TileContext(nc) as tc:
        _kernel_body(tc, ptdr, pb16, xr, rl, rowsum_o, diag_o, colsum_o, partials_o)
    nc.compile()
    return nc


_NC_CACHE = None


def _get_nc():
    global _NC_CACHE
    if _NC_CACHE is None:
        _NC_CACHE = _build()
    return _NC_CACHE


def _prep_core(P, c):
    ploc = np.roll(P, -c * RPC, axis=0)[:PROJ_ROWS]
    pt = (ploc.T * FP8_SCALE).astype(F8)
    ptdr = np.ascontiguousarray(pt.reshape(64, 2, PROJ_ROWS))
    pb16 = np.ascontiguousarray(ploc[:RPC].T.astype(BF))
    return ptdr, pb16


def _run(projections, xrecon, recon_label, trace=False, **spmd_kwargs):
    nc = _get_nc()
    P = np.ascontiguousarray(np.asarray(projections, dtype=np.float32))
    XR = np.asarray(xrecon, dtype=np.float32).astype(BF)
    RL = np.asarray(recon_label, dtype=np.float32).astype(BF)
    in_maps = []
    for c in range(NCORES):
        ptdr, pb16 = _prep_core(P, c)
        in_maps.append(
            {
                "ptdr": ptdr,
                "pb16": pb16,
                "xr": np.ascontiguousarray(XR[c * RPC : (c + 1) * RPC]),
                "rl": np.ascontiguousarray(RL[c * RPC : (c + 1) * RPC]),
            }
        )
    return run_bass_kernel_spmd(
        nc, in_maps, core_ids=list(range(NCORES)), trace=trace, **spmd_kwargs
    )


_MASKF = np.kron(np.eye(32), np.ones((4, 4)))


def _combine(results):
    rowsum = np.zeros(N, np.float64)
    possum = np.zeros(N, np.float64)
    for c in range(NCORES):
        base = c * RPC
        rowsum[base : base + RPC] += results[c]["rowsum_o"].T.reshape(-1).astype(np.float64)
        diag = results[c]["diag_o"].astype(np.float64).reshape(RT, 128, 128)
        ps = (diag * _MASKF[None]).sum(-1)  # [RT, 128]
        possum[base : base + RPC] += ps.reshape(-1)
        cs = results[c]["colsum_o"].astype(np.float64).reshape(7, 512)
        for k, (ck, ntiles) in enumerate(COL_CHUNKS):
            gidx = (base + ck + np.arange(512)) % N
            rowsum[gidx] += cs[k]
    recon_ss = 0.0
    A = 0.0
    B = 0.0
    for c in range(NCORES):
        p = results[c]["partials_o"].astype(np.float64)
        recon_ss += p[:, 0].sum()
        A += p[:, 1].sum()
        B += p[:, 2].sum()
    closs = float(np.mean(np.log(rowsum) - np.log(possum)))
    recon_loss = recon_ss / (N * F)
    dist_loss = (4.0 * A - B) / ((N // 4) * 6 * D)
    loss = closs + recon_loss + dist_loss
    return (
        np.float32(loss),
        np.float32(closs),
        np.float32(recon_loss),
        np.float32(dist_loss),
    )


def kernel(projections, xrecon, recon_label):
    br = _run(projections, xrecon, recon_label)
    return _combine(br.results)


# revision 48
# speedup vs baseline: 1.0350x; 1.0350x over previous
"""Trainium2 Bass kernel for nn_JointLoss (recon MSE + SimCLR contrastive + group distance loss).

Strategy (symmetric data-parallel over 8 NeuronCores):
  - exp(sim) dominates: ACT is the only engine with exp, 1 col/cycle @1.2GHz.
    The sim matrix is symmetric: each core evaluates its 1024 rows vs column
    groups c..c+3 (full) plus 3 quarters of the c+4 pair block -- 38,912 cols
    instead of 65,536.  Transposed entries are recovered as column sums.
  - Host pre-transposes projections into the fp8e4m3 DoubleRow layout
    [64, 2, 5120] (x16): no on-chip transposes, sim matmuls at 0.5 cyc/col.
  - 24 activations (PSUM regions A/B = [0:1792]/[1792:3584] ping-pong) write
    exp to SBUF bf16 with accum_out row-sums.
  - Column-sum tile-accumulation over [1024:3584] runs as gpsimd
    DMA-accumulate transfers on the otherwise-idle DMA path; the PE then does
    one ones-matmul per 512-col chunk.  The [3584:4096] and [4608:5120]
    pieces accumulate with a few small DVE adds.
  - possum (masked diagonal sums) moves to the host via tiny diag-block DMA
    dumps; recon-MSE is GpSimd subtract+square with DVE reduces; distance
    partials on DVE.  This keeps the DVE stream light -- its real throughput
    is ~2x below the scheduler's model and heavy DVE work causes cross-engine
    stalls.
  - Host finishes in float64 from tiny outputs.
"""

import os
import sys

if "/opt/trn_rl_repo" not in sys.path:
    sys.path.insert(0, "/opt/trn_rl_repo")

from contextlib import ExitStack

import numpy as np
import ml_dtypes

import concourse.bacc as bacc
import concourse.tile as tile
from concourse import mybir
from concourse.bass_utils import run_bass_kernel_spmd

N = 8192
D = 128
F = 784
NCORES = 8
RPC = N // NCORES
RT = RPC // 128
PROJ_ROWS = 5 * RPC
TAU = 0.1
FP8_SCALE = 16.0

f32 = mybir.dt.float32
bf16 = mybir.dt.bfloat16
fp8 = mybir.dt.float8e4
BF = ml_dtypes.bfloat16
F8 = ml_dtypes.float8_e4m3fn

COL_CHUNKS = [
    (1024, 8), (1536, 8), (2048, 8), (2560, 8), (3072, 8),  # from accA
    (3584, 8),                                              # from sa5
    (4608, 4),                                              # from sa6 (tiles 0-3)
]

GB0, GB1, GB2 = 1792, 3584, 5120


def _bank_splits(a, b):
    cuts = [a]
    nxt = (a // 512 + 1) * 512
    while nxt < b:
        cuts.append(nxt)
        nxt += 512
    cuts.append(b)
    return list(zip(cuts[:-1], cuts[1:]))


def _kernel_body(tc, ptdr, pb16, xr, rl, rowsum_o, diag_o, colsum_o, partials_o):
    nc = tc.nc
    AX = mybir.AxisListType
    ALU = mybir.AluOpType
    EXP = mybir.ActivationFunctionType.Exp
    DR = mybir.MatmulPerfMode.DoubleRow
    with ExitStack() as ctx:
        consts = ctx.enter_context(tc.tile_pool(name="consts", bufs=1))
        big = ctx.enter_context(tc.tile_pool(name="big", bufs=1))
        stats = ctx.enter_context(tc.tile_pool(name="stats", bufs=1))
        psum = ctx.enter_context(tc.tile_pool(name="psum", bufs=1, space="PSUM"))

        pt_dr = big.tile([64, 2, PROJ_ROWS], fp8)
        nc.sync.dma_start(pt_dr[:, :, 0:GB0], ptdr[:, :, 0:GB0])
        nc.sync.dma_start(pt_dr[:, :, GB0:GB1], ptdr[:, :, GB0:GB1])
        nc.sync.dma_start(pt_dr[:, :, GB1:GB2], ptdr[:, :, GB1:GB2])
        pb = consts.tile([128, RPC], bf16)
        nc.sync.dma_start(pb, pb16)

        exp_sbuf = big.tile([128, RT, PROJ_ROWS], bf16)
        xr_sb = big.tile([128, RT, F], bf16)
        rl_sb = big.tile([128, RT, F], bf16)
        nc.sync.dma_start(xr_sb[:, 0:4, :], xr[0:512].rearrange("(t p) j -> p t j", p=128))
        nc.sync.dma_start(rl_sb[:, 0:4, :], rl[0:512].rearrange("(t p) j -> p t j", p=128))
        nc.sync.dma_start(xr_sb[:, 4:8, :], xr[512:1024].rearrange("(t p) j -> p t j", p=128))
        nc.sync.dma_start(rl_sb[:, 4:8, :], rl[512:1024].rearrange("(t p) j -> p t j", p=128))

        ones_bf = consts.tile([128, 1], bf16)
        nc.vector.memset(ones_bf, 1.0)

        rparts = stats.tile([128, RT, 3], f32)
        rowsum_sb = stats.tile([128, RT], f32)
        recon_parts = stats.tile([128, RT], f32)
        stats4 = stats.tile([128, 4], f32)
        s_groups = stats.tile([128, RPC // 4], f32)
        junk_bf = stats.tile([128, RPC], bf16)
        junkf = stats.tile([128, RPC // 4], f32)
        d_bf = stats.tile([128, F], bf16)
        d2_all = stats.tile([128, RT, F], bf16)
        colstage = stats.tile([128, 7, 512], f32)
        accA = stats.tile([128, 2560], bf16)   # tile sums over cols [1024:3584]
        accB = stats.tile([128, 2560], bf16)
        accY = stats.tile([128, 2560], bf16)
        sa5 = stats.tile([128, 512], bf16)
        sb5 = stats.tile([128, 512], bf16)
        y5 = stats.tile([128, 512], bf16)
        sa6 = stats.tile([128, 512], bf16)
        y6 = stats.tile([128, 512], bf16)

        pacc = psum.tile([128, 4096], f32)
        REG = [0, 1792]

        def colsum_mm(src):
            nc.tensor.matmul(
                pacc[0:1, 3584:4096], ones_bf[:, 0:1], src, start=True, stop=True
            )

        def colsum_drain(k):
            nc.vector.tensor_copy(colstage[0:1, k, :], pacc[0:1, 3584:4096])
            nc.sync.dma_start(colsum_o[k], colstage[0:1, k, :])

        def sim_act(t, au, g):
            reg = REG[au % 2]
            w = pt_dr[:, :, t * 128 : (t + 1) * 128]
            if g == 0:
                cols = [(0, GB0)]
            elif g == 1:
                cols = [(GB0, GB1)]
            elif t < 4:
                cols = [(GB1, GB2)]
            else:
                cols = [(GB1, 4096), (4608, 5120)]
            off = 0
            for c0, c1 in cols:
                for p0, p1 in _bank_splits(reg + off, reg + off + (c1 - c0)):
                    s0 = c0 + (p0 - reg - off)
                    nc.tensor.matmul(
                        pacc[:, p0:p1], w, pt_dr[:, :, s0 : s0 + (p1 - p0)],
                        start=True, stop=True, perf_mode=DR,
                    )
                off += c1 - c0
            if g == 0:
                o0, o1 = 0, GB0
            elif g == 1:
                o0, o1 = GB0, GB1
            elif t < 4:
                o0, o1 = GB1, GB2
            else:
                o0, o1 = GB1, 4608
            nc.scalar.activation(
                exp_sbuf[:, t, o0:o1],
                pacc[:, reg : reg + (o1 - o0)],
                EXP,
                scale=1.0 / (TAU * FP8_SCALE * FP8_SCALE),
                accum_out=rparts[:, t, g : g + 1],
            )

        # distance-loss partials (DVE) from own rows -- first in the DVE queue
        nc.vector.reduce_sum(s_groups, pb.rearrange("p (g s) -> p g s", s=4), axis=AX.X)
        nc.vector.affine_mul_reduce(
            out=junk_bf, accum_out=stats4[:, 1:2], in0=pb, in1=pb, scale=1.0, bias=0.0
        )
        nc.vector.tensor_tensor(junkf, s_groups, s_groups, ALU.mult)
        nc.vector.reduce_sum(stats4[:, 2:3], junkf, axis=AX.X)
        nc.vector.memset(stats4[:, 3:4], 0.0)

        au = 0
        E = lambda t, a, b: exp_sbuf[:, t, a:b]

        # ---- G0 pass: cols [0:1792]; stream remaining input DMAs; diag dumps ----
        for t in range(RT):
            sim_act(t, au, 0)
            au += 1
            nc.sync.dma_start(diag_o[t], exp_sbuf[:, t, t * 128 : (t + 1) * 128])

        # ---- G1 pass: cols [1792:3584]; DVE tile-sum tree [1024:3584]; MSE ----
        for t in range(RT):
            sim_act(t, au, 1)
            au += 1
            # DVE tree accumulation of exp tiles over cols [1024:3584]
            if t == 1:
                nc.vector.tensor_tensor(accA, E(0, 1024, 3584), E(1, 1024, 3584), ALU.add)
            if t == 3:
                nc.vector.tensor_tensor(accY, E(2, 1024, 3584), E(3, 1024, 3584), ALU.add)
                nc.vector.tensor_tensor(accA, accA, accY, ALU.add)
            if t == 5:
                nc.vector.tensor_tensor(accB, E(4, 1024, 3584), E(5, 1024, 3584), ALU.add)
            if t == 7:
                nc.vector.tensor_tensor(accY, E(6, 1024, 3584), E(7, 1024, 3584), ALU.add)
                nc.vector.tensor_tensor(accB, accB, accY, ALU.add)
            nc.gpsimd.tensor_tensor(d_bf, xr_sb[:, t, :], rl_sb[:, t, :], ALU.subtract)
            nc.gpsimd.tensor_tensor(d2_all[:, t, :], d_bf, d_bf, ALU.mult)

        # ---- G2 pass: cols [3584:5120] (packed t>=4); colsum mms; MSE reduce ----
        for t in range(RT):
            sim_act(t, au, 2)
            au += 1
            if t == 0:
                nc.vector.tensor_tensor(accA, accA, accB, ALU.add)
            if t == 1:
                nc.vector.tensor_tensor(sa5, E(0, 3584, 4096), E(1, 3584, 4096), ALU.add)
            if t == 3:
                nc.vector.tensor_tensor(y5, E(2, 3584, 4096), E(3, 3584, 4096), ALU.add)
                nc.vector.tensor_tensor(sa5, sa5, y5, ALU.add)
                nc.vector.tensor_tensor(sa6, E(0, 4608, 5120), E(1, 4608, 5120), ALU.add)
                nc.vector.tensor_tensor(y6, E(2, 4608, 5120), E(3, 4608, 5120), ALU.add)
                nc.vector.tensor_tensor(sa6, sa6, y6, ALU.add)
            if t == 5:
                nc.vector.tensor_tensor(sb5, E(4, 3584, 4096), E(5, 3584, 4096), ALU.add)
            if t == 7:
                nc.vector.tensor_tensor(y5, E(6, 3584, 4096), E(7, 3584, 4096), ALU.add)
                nc.vector.tensor_tensor(sb5, sb5, y5, ALU.add)
                nc.vector.tensor_tensor(sa5, sa5, sb5, ALU.add)
            # chunk mms: accA chunks at steps 1-5, chunk6 at t6; each step
            # drains the previous chunk BEFORE the next mm reuses the slot
            if 2 <= t <= 6:
                colsum_drain(t - 2)
            if 1 <= t <= 5:
                colsum_mm(accA[:, (t - 1) * 512 : t * 512])
            if t == 6:
                colsum_mm(sa6)
            if t == 7:
                colsum_drain(6)
            # MSE reduce of tile t
            nc.vector.reduce_sum(recon_parts[:, t : t + 1], d2_all[:, t, :], axis=AX.X)

        # epilogue: chunk 5
        colsum_mm(sa5)
        colsum_drain(5)

        nc.vector.reduce_sum(rowsum_sb, rparts, axis=AX.X)
        nc.vector.reduce_sum(stats4[:, 0:1], recon_parts, axis=AX.X)

        nc.sync.dma_start(rowsum_o, rowsum_sb)
        nc.sync.dma_start(partials_o, stats4)


def _build():
    nc = bacc.Bacc("TRN2", target_bir_lowering=False, debug=False, num_devices=NCORES)
    ptdr = nc.dram_tensor("ptdr", [64, 2, PROJ_ROWS], fp8, kind="ExternalInput").ap()
    pb16 = nc.dram_tensor("pb16", [128, RPC], bf16, kind="ExternalInput").ap()
    xr = nc.dram_tensor("xr", [RPC, F], bf16, kind="ExternalInput").ap()
    rl = nc.dram_tensor("rl", [RPC, F], bf16, kind="ExternalInput").ap()
    rowsum_o = nc.dram_tensor("rowsum_o", [128, RT], f32, kind="ExternalOutput").ap()
    diag_o = nc.dram_tensor("diag_o", [RT, 128, 128], bf16, kind="ExternalOutput").ap()
    colsum_o = nc.dram_tensor("colsum_o", [7, 512], f32, kind="ExternalOutput").ap()
    partials_o = nc.dram_tensor("partials_o", [128, 4], f32, kind="ExternalOutput").ap()

    with tile.TileContext(nc) as tc:
        _kernel_body(tc, ptdr, pb16, xr, rl, rowsum_o, diag_o, colsum_o, partials_o)
    nc.compile()
    return nc


_NC_CACHE = None


def _get_nc():
    global _NC_CACHE
    if _NC_CACHE is None:
        _NC_CACHE = _build()
    return _NC_CACHE


def _prep_core(P, c):
    ploc = np.roll(P, -c * RPC, axis=0)[:PROJ_ROWS]
    pt = (ploc.T * FP8_SCALE).astype(F8)
    ptdr = np.ascontiguousarray(pt.reshape(64, 2, PROJ_ROWS))
    pb16 = np.ascontiguousarray(ploc[:RPC].T.astype(BF))
    return ptdr, pb16


def _run(projections, xrecon, recon_label, trace=False, **spmd_kwargs):
    nc = _get_nc()
    P = np.ascontiguousarray(np.asarray(projections, dtype=np.float32))
    XR = np.asarray(xrecon, dtype=np.float32).astype(BF)
    RL = np.asarray(recon_label, dtype=np.float32).astype(BF)
    in_maps = []
    for c in range(NCORES):
        ptdr, pb16 = _prep_core(P, c)
        in_maps.append(
            {
                "ptdr": ptdr,
                "pb16": pb16,
                "xr": np.ascontiguousarray(XR[c * RPC : (c + 1) * RPC]),
                "rl": np.ascontiguousarray(RL[c * RPC : (c + 1) * RPC]),
            }
        )
    return run_bass_kernel_spmd(
        nc, in_maps, core_ids=list(range(NCORES)), trace=trace, **spmd_kwargs
    )


_MASKF = np.kron(np.eye(32), np.ones((4, 4)))


def _combine(results):
    rowsum = np.zeros(N, np.float64)
    possum = np.zeros(N, np.float64)
    for c in range(NCORES):
        base = c * RPC
        rowsum[base : base + RPC] += results[c]["rowsum_o"].T.reshape(-1).astype(np.float64)
        diag = results[c]["diag_o"].astype(np.float64).reshape(RT, 128, 128)
        ps = (diag * _MASKF[None]).sum(-1)  # [RT, 128]
        possum[base : base + RPC] += ps.reshape(-1)
        cs = results[c]["colsum_o"].astype(np.float64).reshape(7, 512)
        for k, (ck, ntiles) in enumerate(COL_CHUNKS):
            gidx = (base + ck + np.arange(512)) % N
            rowsum[gidx] += cs[k]
    recon_ss = 0.0
    A = 0.0
    B = 0.0
    for c in range(NCORES):
        p = results[c]["partials_o"].astype(np.float64)
        recon_ss += p[:, 0].sum()
        A += p[:, 1].sum()
        B += p[:, 2].sum()
    closs = float(np.mean(np.log(rowsum) - np.log(possum)))
    recon_loss = recon_ss / (N * F)
    dist_loss = (4.0 * A - B) / ((N // 4) * 6 * D)
    loss = closs + recon_loss + dist_loss
    return (
        np.float32(loss),
        np.float32(closs),
        np.float32(recon_loss),
        np.float32(dist_loss),
    )


def kernel(projections, xrecon, recon_label):
    br = _run(projections, xrecon, recon_label)
    return _combine(br.results)
